# revision 1
# baseline (speedup 1.0000x reference)
"""Multi-head causal self-attention (GPT-style block) on 8 Trainium2 NeuronCores.

Strategy: data-parallel over batch (B=8 -> 1 batch element per core), weights
replicated. Per-core dataflow keeps everything "transposed" so no operand ever
needs an extra transpose beyond x itself:

  x [T,H] --PE transpose--> xT [H,T] (bf16)
  qT/kT [n,T] = W_attn[:, n].T-stationary matmuls over xT   (n on partitions)
  v    [T,n] = xT-stationary matmuls over W_attn[:, v-part] (T on partitions)
  scores^T [k,q] = kT_h.T @ qT_h  (K=64 contraction; even/odd heads at
                   partition bases 0/64 -> concurrent PE row-groups)
  P^T = exp(0.125*scores^T) via ACT, causal diag tiles masked by upper-tri mult
  out^T[d,q] & softmax denom = [v_h | ones].T @ P^T  (ones col -> denom row)
  normalize via PE-broadcast of 1/denom, DVE multiply
  y [T,H] = out^T-stationary matmuls over W_proj + bias
All matmul compute in bf16 with fp32 PSUM accumulation.
"""

import numpy as np

import concourse.bass as bass
import concourse.mybir as mybir
import concourse.tile as tile
from concourse import bacc, bass_utils
from concourse.masks import make_identity, make_upper_triangular

F32 = mybir.dt.float32
BF16 = mybir.dt.bfloat16

T = 1024   # tokens per batch element
H = 768    # hidden
NH = 12    # heads
HS = 64    # head size
TT = T // 128   # token tiles (8)
FT = H // 128   # feature tiles (6)
N_CORES = 8


def build():
    nc = bacc.Bacc(None, target_bir_lowering=False)

    x_d = nc.dram_tensor("x", [T, H], BF16, kind="ExternalInput")
    wa_d = nc.dram_tensor("W_attn", [H, 3 * H], BF16, kind="ExternalInput")
    ba_d = nc.dram_tensor("b_attn", [3 * H], F32, kind="ExternalInput")
    wp_d = nc.dram_tensor("W_proj", [H, H], BF16, kind="ExternalInput")
    bp_d = nc.dram_tensor("b_proj", [H], F32, kind="ExternalInput")
    y_d = nc.dram_tensor("y", [T, H], F32, kind="ExternalOutput")

    with tile.TileContext(nc) as tc:
        with (
            tc.tile_pool(name="sb", bufs=1) as sb,
            tc.tile_pool(name="ps", bufs=1, space="PSUM") as ps,
        ):
            # ---------------- persistent SBUF tensors ----------------
            wat = sb.tile([128, FT, 3 * H], BF16, tag="wat")        # W_attn bf16, k-tiled
            wpr = sb.tile([128, FT, H], BF16, tag="wpr")            # W_proj bf16
            x_bf = sb.tile([128, TT, H], BF16, tag="x_bf")          # x bf16, token-tiled
            xT = sb.tile([128, FT, T], BF16, tag="xT")              # x^T bf16, feat-tiled
            kT = sb.tile([128, NH // 2, T], BF16, tag="kT")         # k^T head pairs
            # q^T zero-padded per head: head h occupies rows 64*(h%2)..+64, rest 0
            qTp = sb.tile([128, NH, T], BF16, tag="qTp")
            v_bf = sb.tile([128, TT, NH * (HS + 1) + 64], BF16, tag="v_bf")  # [v|1] per head + pad
            oT = sb.tile([128, FT, T], BF16, tag="oT")              # attn out^T
            bcols = sb.tile([128, 12], F32, tag="bcols")            # b_attn[:1536] as columns
            ba_row = sb.tile([1, 3 * H], F32, tag="ba_row")
            ba_rowb = sb.tile([1, 3 * H], BF16, tag="ba_rowb")
            bp_row = sb.tile([1, H], F32, tag="bp_row")
            bp_rowb = sb.tile([1, H], BF16, tag="bp_rowb")
            ones0 = sb.tile([1, 128], BF16, tag="ones0")            # ones at partition 0
            ones64 = sb.tile([65, 128], BF16, tag="ones64")         # row 64 = ones
            ones64f = sb.tile([65, 128], F32, tag="ones64f")        # f32 ones row (f32r bcast)
            tri = sb.tile([128, 128], BF16, tag="tri")              # upper-tri (p<=f) of 1.0
            ident = sb.tile([128, 128], BF16, tag="ident")

            # ---------------- constants / small loads ----------------
            make_identity(nc, ident[:])
            make_upper_triangular(nc, tri[:], val=1.0, diag=True)
            nc.gpsimd.memset(ones0[:], 1.0)
            nc.gpsimd.memset(ones64[64:65, :], 1.0)
            nc.gpsimd.memset(ones64f[64:65, :], 1.0)
            nc.gpsimd.memset(qTp[:], 0.0)
            nc.gpsimd.memset(v_bf[:, :, 12 * (HS + 1):], 0.0)       # tail pad
            nc.gpsimd.memset(v_bf[:, :, HS:12 * (HS + 1):HS + 1], 1.0)  # ones cols

            # ---------------- load x, cast, transpose (x DMAs lead the queue) ----------------
            with nc.named_scope("xT"):
                for tt in range(TT):
                    nc.sync.dma_start(x_bf[:, tt, :], x_d[tt * 128:(tt + 1) * 128, :])
                    pt = ps.tile([128, FT * 128], BF16, tag="op", bufs=3)
                    for ft in range(FT):
                        nc.tensor.transpose(
                            pt[:, ft * 128:(ft + 1) * 128],
                            x_bf[:, tt, ft * 128:(ft + 1) * 128],
                            ident[:],
                        )
                    nc.vector.tensor_copy(
                        xT[:, :, tt * 128:(tt + 1) * 128],
                        pt[:].rearrange("p (f t) -> p f t", t=128),
                    )

            # bias loads (needed from QKV evac onward)
            nc.sync.dma_start(bcols[:], ba_d[: 12 * 128].rearrange("(t p) -> p t", p=128))
            nc.sync.dma_start(ba_row[:], ba_d[None, :])
            nc.sync.dma_start(bp_row[:], bp_d[None, :])

            # ---------------- load W_attn, cast ----------------
            # q/k columns first in 256-col blocks so QKV can start early
            for ft in range(FT):
                nc.sync.dma_start(wat[:, ft, :2 * H], wa_d[ft * 128:(ft + 1) * 128, :2 * H])
            for ft in range(FT):
                nc.sync.dma_start(wat[:, ft, 2 * H:], wa_d[ft * 128:(ft + 1) * 128, 2 * H:])

            nc.vector.tensor_copy(ba_rowb[:], ba_row[:])
            nc.vector.tensor_copy(bp_rowb[:], bp_row[:])

            # ---------------- QKV projection ----------------
            with nc.named_scope("qkv"):
                # q^T / k^T : [n-tile, token] with n on partitions.
                # Both token-groups inner so consecutive matmuls share lhsT.
                for nt in range(NH):
                    pqs = [
                        ps.tile([128, 512], F32, tag="op", bufs=3, name="pq0"),
                        ps.tile([128, 512], F32, tag="op", bufs=3, name="pq1"),
                    ]
                    for ft in range(FT):
                        for tg in range(2):
                            nc.tensor.matmul(
                                pqs[tg][:],
                                wat[:, ft, nt * 128:(nt + 1) * 128],
                                xT[:, ft, tg * 512:(tg + 1) * 512],
                                start=(ft == 0),
                                stop=(ft == FT - 1),
                            )
                    for tg in range(2):
                        sl = slice(tg * 512, (tg + 1) * 512)
                        ident_fn = mybir.ActivationFunctionType.Identity
                        if nt < 6:  # q: split halves into per-head zero-padded tiles
                            nc.scalar.activation(
                                qTp[:64, 2 * nt, sl], pqs[tg][:64, :], ident_fn,
                                bias=bcols[:64, nt:nt + 1])
                            nc.scalar.activation(
                                qTp[64:, 2 * nt + 1, sl], pqs[tg][64:, :], ident_fn,
                                bias=bcols[64:, nt:nt + 1])
                        else:       # k: keep head-pair tiles
                            nc.scalar.activation(
                                kT[:, nt - 6, sl], pqs[tg][:], ident_fn,
                                bias=bcols[:, nt:nt + 1])
                # v : [token, n] natural, bias added via K=1 ones-matmul
                for tt in range(TT):
                    pvs = [
                        ps.tile([128, 512], F32, tag="op", bufs=3, name="pv0"),
                        ps.tile([128, 512], F32, tag="op", bufs=3, name="pv1"),
                    ]
                    for ft in range(FT):
                        for ng in range(2):
                            w = 512 if ng == 0 else 256
                            nc.tensor.matmul(
                                pvs[ng][:, :w],
                                xT[:, ft, tt * 128:(tt + 1) * 128],
                                wat[:, ft, 2 * H + ng * 512: 2 * H + ng * 512 + w],
                                start=(ft == 0),
                                stop=False,
                            )
                    for ng in range(2):
                        w = 512 if ng == 0 else 256
                        nc.tensor.matmul(
                            pvs[ng][:, :w],
                            ones0[:1, :],
                            ba_rowb[:1, 2 * H + ng * 512: 2 * H + ng * 512 + w],
                            start=False,
                            stop=True,
                        )
                        hlo = ng * 8
                        hhi = 8 if ng == 0 else 12
                        v3 = v_bf[:, tt, :12 * (HS + 1)].rearrange("p (h c) -> p h c", c=HS + 1)
                        nc.scalar.copy(
                            v3[:, hlo:hhi, :HS],
                            pvs[ng][:, :w].rearrange("p (h d) -> p h d", d=HS),
                        )

            # W_proj loads emitted here: only needed by proj, keeps DVE free early
            for ft in range(FT):
                nc.sync.dma_start(wpr[:, ft, :], wp_d[ft * 128:(ft + 1) * 128, :])

            # ---------------- attention (per head pair) ----------------
            # The normalize chain (recip -> broadcast-matmul -> multiply) of
            # each group is deferred by one group so the PE never head-of-line
            # blocks on the DVE reciprocal: PE stream per group is
            #   [scores (even/odd row-group pairs)] [prev group's bcasts] [AV]
            def norm_flush(pending):
                for hi, hp_, qg_, op_, recb_ in pending:
                    base = 64 * hi
                    bp = ps.tile([128, 512], F32, tag="bc", bufs=1, name="bp")
                    nc.tensor.matmul(
                        bp[:], ones64[64:65, :], recb_[64:65, :],
                        start=True, stop=True,
                    )
                    bpb = sb.tile([64, 512], BF16, tag="bpb", bufs=2, name="bpb")
                    nc.vector.tensor_copy(bpb[:], bp[:64, :])
                    dst = slice(512 * qg_, 512 * (qg_ + 1))
                    if hi == 0:
                        nc.vector.tensor_mul(oT[:64, hp_, dst], op_[:64, :], bpb[:])
                    else:
                        sc = sb.tile([64, 512], BF16, tag="sc", bufs=3, name="sc")
                        nc.vector.tensor_mul(sc[:], op_[:64, :], bpb[:])
                        nc.sync.dma_start(oT[base:base + 64, hp_, dst], sc[:])

            with nc.named_scope("attn"):
                pending = []
                for hp in range(NH // 2):
                    for qg in range(2):
                        kts = list(range(4 * qg + 4))
                        pts = []
                        for hi in range(2):
                            pts.append(sb.tile([128, 8, 512], BF16, tag=f"pT{hi}", bufs=2, name=f"pt{hi}"))
                        for kp in range(0, len(kts), 2):
                            kt0, kt1 = kts[kp], kts[kp + 1]
                            offs, ws = [], []
                            for j, kt in enumerate((kt0, kt1)):
                                q_off = max(128 * kt, 512 * qg)
                                offs.append(q_off)
                                ws.append(512 * (qg + 1) - q_off)
                            vw = 512 + ws[1]  # exp span: slot0 prefix + slot1 valid part
                            sps2 = [
                                ps.tile([128, 1024], F32, tag="sp0", bufs=1, name="spA"),
                                ps.tile([128, 1024], F32, tag="sp1", bufs=1, name="spB"),
                            ]
                            for j, kt in enumerate((kt0, kt1)):
                                for hi in range(2):  # same lhsT back-to-back
                                    nc.tensor.matmul(
                                        sps2[hi][:, j * 512:j * 512 + ws[j]],
                                        kT[:, hp, kt * 128:(kt + 1) * 128],
                                        qTp[:, 2 * hp + hi, offs[j]:offs[j] + ws[j]],
                                        start=True,
                                        stop=True,
                                    )
                            for hi in range(2):
                                dst = pts[hi][:, kt0:kt0 + 2, :].rearrange("p a b -> p (a b)")
                                nc.scalar.activation(
                                    dst[:, :vw], sps2[hi][:, :vw],
                                    mybir.ActivationFunctionType.Exp, scale=0.125,
                                )
                                if 128 * kt0 >= 512 * qg:  # diagonal tiles: causal mask
                                    nc.gpsimd.tensor_mul(
                                        pts[hi][:, kt0, :128], pts[hi][:, kt0, :128], tri[:])
                                    nc.gpsimd.tensor_mul(
                                        pts[hi][:, kt1, :128], pts[hi][:, kt1, :128], tri[:])
                        norm_flush(pending)
                        pending = []
                        for hi in range(2):
                            h = 2 * hp + hi
                            op = ps.tile([128, 512], F32, tag="op", bufs=3)
                            for j, kt in enumerate(kts):
                                q_off = max(128 * kt, 512 * qg)
                                w = 512 * (qg + 1) - q_off
                                off = q_off - 512 * qg
                                nc.tensor.matmul(
                                    op[:, off:off + w],
                                    v_bf[:, kt, 65 * h:65 * h + 128],
                                    pts[hi][:, kt, :w],
                                    start=(j == 0),
                                    stop=(j == len(kts) - 1),
                                )
                            # reciprocal of denominator (row 64), stays on partition 64
                            rec = sb.tile([65, 512], F32, tag="rec", bufs=2)
                            recb = sb.tile([65, 512], BF16, tag="recb", bufs=2)
                            nc.vector.reciprocal_approx_fast(rec[:, :], op[:65, :])
                            nc.vector.tensor_copy(recb[64:65, :], rec[64:65, :])
                            pending.append((hi, hp, qg, op, recb))
                norm_flush(pending)

            # ---------------- output projection ----------------
            with nc.named_scope("proj"):
                for tt in range(TT):
                    ysb = sb.tile([128, H], F32, tag="ysb", bufs=2)
                    pys = [
                        ps.tile([128, 512], F32, tag="op", bufs=3, name="py0"),
                        ps.tile([128, 512], F32, tag="op", bufs=3, name="py1"),
                    ]
                    for ft in range(FT):
                        for ng in range(2):
                            w = 512 if ng == 0 else 256
                            nc.tensor.matmul(
                                pys[ng][:, :w],
                                oT[:, ft, tt * 128:(tt + 1) * 128],
                                wpr[:, ft, ng * 512:ng * 512 + w],
                                start=(ft == 0),
                                stop=False,
                            )
                    for ng in range(2):
                        w = 512 if ng == 0 else 256
                        nc.tensor.matmul(
                            pys[ng][:, :w],
                            ones0[:1, :],
                            bp_rowb[:1, ng * 512:ng * 512 + w],
                            start=False,
                            stop=True,
                        )
                        nc.scalar.copy(ysb[:, ng * 512:ng * 512 + w], pys[ng][:, :w])
                    nc.sync.dma_start(y_d[tt * 128:(tt + 1) * 128, :], ysb[:])

    nc.compile()
    return nc


_NC = None


def _run(in_maps, trace=False, **kwargs):
    global _NC
    if _NC is None:
        _NC = build()
    return bass_utils.run_bass_kernel_spmd(
        _NC, in_maps, core_ids=list(range(N_CORES)), trace=trace, **kwargs
    )


def make_in_maps(x, W_attn, b_attn, W_proj, b_proj):
    import ml_dtypes
    bf = ml_dtypes.bfloat16
    x = np.asarray(x, dtype=np.float32).astype(bf)
    W_attn = np.ascontiguousarray(np.asarray(W_attn, dtype=np.float32).astype(bf))
    b_attn = np.ascontiguousarray(np.asarray(b_attn, dtype=np.float32))
    W_proj = np.ascontiguousarray(np.asarray(W_proj, dtype=np.float32).astype(bf))
    b_proj = np.ascontiguousarray(np.asarray(b_proj, dtype=np.float32))
    return [
        {
            "x": np.ascontiguousarray(x[b]),
            "W_attn": W_attn,
            "b_attn": b_attn,
            "W_proj": W_proj,
            "b_proj": b_proj,
        }
        for b in range(N_CORES)
    ]


def kernel(x, W_attn, b_attn, W_proj, b_proj):
    in_maps = make_in_maps(x, W_attn, b_attn, W_proj, b_proj)
    res = _run(in_maps, trace=False)
    return np.stack([res.results[b]["y"] for b in range(N_CORES)]).astype(np.float32)



# revision 37
# speedup vs baseline: 1.1019x; 1.1019x over previous
"""Multi-head causal self-attention (GPT-style block) on 8 Trainium2 NeuronCores.

Strategy: data-parallel over batch (B=8 -> 1 batch element per core), weights
replicated. Per-core dataflow keeps everything "transposed" so no operand ever
needs an on-chip transpose:

  xT [H,T] bf16 and x8T [H,T] fp8e4 (x16) shipped pre-transposed from host
  q/k projections: fp8 DoubleRow matmuls (W q/k cols pre-scaled x512 fp8,
               K=256 per instruction), evac'd to bf16 by DVE/ACT + bias add
  v    [T,n] = xT-stationary bf16 matmuls over W_attn v-cols
  scores^T [k,q] = bf16 K=64 matmuls on head-parity partition halves
               (no zero padding; tile_position rides partition base 0/64)
  P^T = exp(0.125*scores^T) via ACT, causal diag tiles masked on GpSimd
  out^T[d,q] & softmax denom = [v_h|ones].T @ P^T (ones col -> denom row)
  normalize via PE-broadcast (f32r) of 1/denom, DVE multiply into oT
  y [T,H] = oT-stationary matmuls over W_proj + bias, DVE evac, bf16 out
All remaining matmul compute in bf16 with fp32 PSUM accumulation.
"""

import numpy as np

import concourse.bass as bass
import concourse.mybir as mybir
import concourse.tile as tile
from concourse import bacc, bass_utils
from concourse.masks import make_upper_triangular

F32 = mybir.dt.float32
F32R = mybir.dt.float32r
BF16 = mybir.dt.bfloat16
F8 = mybir.dt.float8e4

T = 1024   # tokens per batch element
H = 768    # hidden
NH = 12    # heads
HS = 64    # head size
TT = T // 128   # token tiles (8)
FT = H // 128   # feature tiles (6)
N_CORES = 8

QK_FP8 = True                   # q/k projection matmuls in fp8 DoubleRow
XS = 16.0                       # fp8 scale for x
WS = 512.0                      # fp8 scale for W_attn q/k columns
EXP_SCALE = 0.125               # folded into the ACT exp
ADD = mybir.AluOpType.add
MULT = mybir.AluOpType.mult
DR = mybir.MatmulPerfMode.DoubleRow
VW = NH * (HS + 1) + 64         # v_bf row width: [v|ones] per head + pad
WCH, WCW = 4, 384               # wa8 chunk count / chunk column width


def build():
    nc = bacc.Bacc(None, target_bir_lowering=False)

    # all inputs pre-tiled on host to [128, FT*cols] so each bulk DMA is
    # 128 contiguous per-partition descriptors
    xT_d = nc.dram_tensor("xt", [128, FT * T], BF16, kind="ExternalInput")
    x8T_d = nc.dram_tensor("x8t", [128, FT * T], F8, kind="ExternalInput")
    waqk_d = nc.dram_tensor("waqkt", [128, FT * 2 * H], BF16, kind="ExternalInput")
    wa8_d = nc.dram_tensor("wa8t", [128, FT * 2 * H], F8, kind="ExternalInput")
    wav_d = nc.dram_tensor("wavt", [128, FT * H], BF16, kind="ExternalInput")
    wp_d = nc.dram_tensor("wprt", [128, FT * H], BF16, kind="ExternalInput")
    ba_d = nc.dram_tensor("b_attn", [3 * H], F32, kind="ExternalInput")
    vb_d = nc.dram_tensor("vbias", [128, H], BF16, kind="ExternalInput")
    pb_d = nc.dram_tensor("pbias", [128, H], BF16, kind="ExternalInput")
    y_d = nc.dram_tensor("y", [T, H], BF16, kind="ExternalOutput")

    with tile.TileContext(nc) as tc:
        with (
            tc.tile_pool(name="sb", bufs=1) as sb,
            tc.tile_pool(name="ps", bufs=1, space="PSUM") as ps,
        ):
            # ---------------- persistent SBUF tensors ----------------
            if QK_FP8:
                wat8 = sb.tile([128, WCH, FT, WCW], F8, tag="wat8")
                xT8 = sb.tile([128, FT, T], F8, tag="xT8")
            else:
                watqk = sb.tile([128, FT, 2 * H], BF16, tag="watqk")
            watv = sb.tile([128, FT, H], BF16, tag="watv")
            wpr = sb.tile([128, FT, H], BF16, tag="wpr")
            xT = sb.tile([128, FT, T], BF16, tag="xT")
            kT = sb.tile([128, FT, T], BF16, tag="kT")   # pair-index major
            qT = sb.tile([128, FT, T], BF16, tag="qT")   # pair-index major
            v_bf = sb.tile([128, TT, VW], BF16, tag="v_bf")  # [v|1] per head
            oT = sb.tile([128, FT, T], BF16, tag="oT")
            bcols = sb.tile([128, 12], F32, tag="bcols")
            vbias = sb.tile([128, H], BF16, tag="vbias")
            pbias = sb.tile([128, H], BF16, tag="pbias")
            onesd = sb.tile([65, 128], BF16, tag="onesd")    # row 64 ones
            tri4 = sb.tile([128, 4, 128], BF16, tag="tri4")  # 4x upper-tri (p<=f)

            # ------- DMA loads (pre-tiled: 128 descriptors per DMA) ---------
            def flat(tile_ap):
                return tile_ap.rearrange("p a b -> p (a b)")

            # q/k matmul operands split across both hwdge queues (concurrent
            # transfers) and chunked so the first QKV matmuls start while the
            # rest is still in flight; everything else follows.  wa8t is
            # shipped chunk-major [128, NC, FT, CW] so chunks stay contiguous.
            if QK_FP8:
                for fp in range(3):
                    nc.scalar.dma_start(
                        flat(xT8[:, 2 * fp:2 * fp + 2, :]),
                        x8T_d[:, fp * 2048:(fp + 1) * 2048])
                for c in range(WCH):
                    nc.sync.dma_start(
                        wat8[:, c, :, :].rearrange("p a b -> p (a b)"),
                        wa8_d[:, c * FT * WCW:(c + 1) * FT * WCW])
            else:
                nc.sync.dma_start(flat(watqk[:]), waqk_d[:, :])
            nc.scalar.dma_start(bcols[:], ba_d[: 12 * 128].rearrange("(t p) -> p t", p=128))
            nc.scalar.dma_start(flat(watv[:]), wav_d[:, :])
            nc.sync.dma_start(flat(xT[:]), xT_d[:, :])
            nc.scalar.dma_start(vbias[:], vb_d[:, :])
            nc.sync.dma_start(wpr[:].rearrange("p a b -> p (a b)"), wp_d[:, :])
            nc.scalar.dma_start(pbias[:], pb_d[:, :])

            # ---------------- constants ----------------
            for r in range(4):
                make_upper_triangular(nc, tri4[:, r, :], val=1.0, diag=True)
            nc.gpsimd.memset(onesd[64:65, :], 1.0)
            nc.gpsimd.memset(v_bf[:, :, NH * (HS + 1):], 0.0)           # tail pad
            nc.gpsimd.memset(v_bf[:, :, HS:NH * (HS + 1):HS + 1], 1.0)  # ones cols

            # ---------------- QKV projection ----------------
            def qk_tile(nt):
                # q^T / k^T : n on partitions; evac'd to fp8 (x QS) on DVE
                pqs = [
                    ps.tile([128, 512], F32, tag="op", bufs=3, name="pq0"),
                    ps.tile([128, 512], F32, tag="op", bufs=3, name="pq1"),
                ]
                if QK_FP8:
                    c, co = divmod(nt * 128, WCW)
                    for fp in range(FT // 2):
                        for tg in range(2):
                            nc.tensor.matmul(
                                pqs[tg][:],
                                wat8[:, c, 2 * fp:2 * fp + 2, co:co + 128],
                                xT8[:, 2 * fp:2 * fp + 2, tg * 512:(tg + 1) * 512],
                                start=(fp == 0),
                                stop=(fp == FT // 2 - 1),
                                perf_mode=DR,
                            )
                else:
                    for ft in range(FT):
                        for tg in range(2):
                            nc.tensor.matmul(
                                pqs[tg][:],
                                watqk[:, ft, nt * 128:(nt + 1) * 128],
                                xT[:, ft, tg * 512:(tg + 1) * 512],
                                start=(ft == 0),
                                stop=(ft == FT - 1),
                            )
                # evac: out = QS*(x@W + b); psum is XS*WS*(x@W) in fp8 mode.
                # Split across DVE (tensor_scalar) and ACT (activation) so
                # neither engine bottlenecks the QKV phase.
                ident_fn = mybir.ActivationFunctionType.Identity
                evac_scale = 1.0 / (XS * WS) if QK_FP8 else 1.0
                for tg in range(2):
                    sl = slice(tg * 512, (tg + 1) * 512)
                    # q/k pair layout: even head rows 0-63, odd rows 64-127
                    dstt = qT if nt < 6 else kT
                    dst = dstt[:, nt % 6, sl]
                    if tg == 0:
                        nc.vector.tensor_scalar(
                            dst, pqs[tg][:],
                            evac_scale, bcols[:, nt:nt + 1], MULT, ADD)
                    else:
                        nc.scalar.activation(
                            dst, pqs[tg][:], ident_fn,
                            bias=bcols[:, nt:nt + 1], scale=evac_scale)

            def v_tile(tt):
                # v : [token, n] natural, bias added via K=1 ones-matmul
                pvs = [
                    ps.tile([128, 512], F32, tag="op", bufs=3, name="pv0"),
                    ps.tile([128, 512], F32, tag="op", bufs=3, name="pv1"),
                ]
                for ft in range(FT):
                    for ng in range(2):
                        w = 512 if ng == 0 else 256
                        nc.tensor.matmul(
                            pvs[ng][:, :w],
                            xT[:, ft, tt * 128:(tt + 1) * 128],
                            watv[:, ft, ng * 512:ng * 512 + w],
                            start=(ft == 0),
                            stop=(ft == FT - 1),
                        )
                v3 = v_bf[:, tt, :NH * (HS + 1)].rearrange("p (h c) -> p h c", c=HS + 1)
                for ng in range(2):
                    w = 512 if ng == 0 else 256
                    hlo = ng * 8
                    hhi = 8 if ng == 0 else 12
                    nc.vector.tensor_add(
                        v3[:, hlo:hhi, :HS],
                        pvs[ng][:, :w].rearrange("p (h d) -> p h d", d=HS),
                        vbias[:, ng * 512:ng * 512 + w].rearrange("p (h d) -> p h d", d=HS),
                    )

            # ---------------- attention + interleaved projection ------------
            # Normalize chain of each group deferred by one group so the PE
            # never head-of-line blocks on the DVE reciprocal.
            def norm_flush(pending):
                for h_, hp_, qg_, op_, recb_ in pending:
                    bp = ps.tile([128, 512], F32, tag="bc", bufs=1, name="bp")
                    nc.tensor.matmul(
                        bp[:], onesd[64:65, :], recb_[64:65, :],
                        start=True, stop=True,
                    )
                    bpb = sb.tile([64, 512], BF16, tag="bpb", bufs=2, name="bpb")
                    nc.vector.tensor_copy(bpb[:], bp[:64, :])
                    dst = slice(512 * qg_, 512 * (qg_ + 1))
                    if h_ % 2 == 0:
                        nc.vector.tensor_mul(
                            oT[:64, hp_, dst], op_[:64, :], bpb[:])
                    else:
                        sc = sb.tile([64, 512], BF16, tag="sc", bufs=3, name="sc")
                        nc.vector.tensor_mul(sc[:], op_[:64, :], bpb[:])
                        nc.sync.dma_start(oT[64:128, hp_, dst], sc[:])

            def emit_scores_pair(qg, hp, pts, kt0, kt1):
                offs, ws = [], []
                for kt in (kt0, kt1):
                    q_off = max(128 * kt, 512 * qg)
                    offs.append(q_off)
                    ws.append(512 * (qg + 1) - q_off)
                vw = 512 + ws[1]  # exp span: slot0 prefix + slot1 valid
                sps2 = [
                    ps.tile([128, 1024], F32, tag="sp0", bufs=1, name="spA"),
                    ps.tile([128, 1024], F32, tag="sp1", bufs=1, name="spB"),
                ]
                for j, kt in enumerate((kt0, kt1)):
                    for hi in range(2):
                        rows = slice(64 * hi, 64 * hi + 64)
                        nc.tensor.matmul(
                            sps2[hi][:, j * 512:j * 512 + ws[j]],
                            kT[rows, hp, kt * 128:(kt + 1) * 128],
                            qT[rows, hp, offs[j]:offs[j] + ws[j]],
                            start=True,
                            stop=True,
                        )
                for hi in range(2):
                    dst = pts[hi][:, kt0:kt0 + 2, :].rearrange("p a b -> p (a b)")
                    nc.scalar.activation(
                        dst[:, :vw], sps2[hi][:, :vw],
                        mybir.ActivationFunctionType.Exp, scale=EXP_SCALE,
                    )

            def proj_tile(tt):
                pys = [
                    ps.tile([128, 512], F32, tag="op", bufs=3, name="py0"),
                    ps.tile([128, 512], F32, tag="op", bufs=3, name="py1"),
                ]
                for ft in range(FT):
                    for ng in range(2):
                        w = 512 if ng == 0 else 256
                        nc.tensor.matmul(
                            pys[ng][:, :w],
                            oT[:, ft, tt * 128:(tt + 1) * 128],
                            wpr[:, ft, ng * 512:ng * 512 + w],
                            start=(ft == 0),
                            stop=(ft == FT - 1),
                        )
                ysb = sb.tile([128, H], BF16, tag="ysb", bufs=2)
                for ng in range(2):
                    w = 512 if ng == 0 else 256
                    nc.vector.tensor_add(
                        ysb[:, ng * 512:ng * 512 + w], pys[ng][:, :w],
                        pbias[:, ng * 512:ng * 512 + w])
                nc.sync.dma_start(y_d[tt * 128:(tt + 1) * 128, :], ysb[:])

            with nc.named_scope("qkv"):
                for nt in range(NH):
                    qk_tile(nt)
                for tt in range(TT):
                    v_tile(tt)

            # Software pipeline over head-pair groups: at step n the PE
            # emits scores(n) interleaved with AV(n-1); normalize of (n-1)
            # flushes at step n+1.  PE never queue-blocks on exp/recip.
            groups = [(0, hp) for hp in range(6)] + [(1, hp) for hp in range(6)]
            pend_flush = []
            prev = None  # (qg, hp, pts) of previous group, AV not yet emitted
            with nc.named_scope("attn"):
                for step in range(len(groups) + 1):
                    cur = groups[step] if step < len(groups) else None
                    norm_flush(pend_flush)
                    pend_flush = []
                    av_list = []
                    if prev is not None:
                        pqg, php, ppts = prev
                        pkts = list(range(4 * pqg + 4))
                        pops = [
                            ps.tile([128, 512], F32, tag="op", bufs=3, name=f"av{hi}")
                            for hi in range(2)
                        ]

                        def av_emit(hi, kt, jk, _pqg=pqg, _php=php, _ppts=ppts,
                                    _pops=pops, _nk=len(pkts)):
                            q_off = max(128 * kt, 512 * _pqg)
                            w = 512 * (_pqg + 1) - q_off
                            off = q_off - 512 * _pqg
                            h = 2 * _php + hi
                            nc.tensor.matmul(
                                _pops[hi][:, off:off + w],
                                v_bf[:, kt, 65 * h:65 * h + 128],
                                _ppts[hi][:, kt, :w],
                                start=(jk == 0),
                                stop=(jk == _nk - 1),
                            )
                        av_list = [(hi, kt, jk)
                                   for hi in range(2)
                                   for jk, kt in enumerate(pkts)]
                    if cur is not None:
                        qg, hp = cur
                        kts = list(range(4 * qg + 4))
                        pts = [
                            sb.tile([128, 8, 512], BF16, tag=f"pT{hi}", bufs=3, name=f"pt{hi}")
                            for hi in range(2)
                        ]
                        pairs = [(kts[i], kts[i + 1]) for i in range(0, len(kts), 2)]
                    else:
                        pairs = []
                    nchunks = max(len(pairs), 1)
                    per = -(-len(av_list) // nchunks) if av_list else 0
                    ai = 0
                    for pi in range(nchunks):
                        if pi < len(pairs):
                            emit_scores_pair(qg, hp, pts, *pairs[pi])
                        for _ in range(per):
                            if ai < len(av_list):
                                av_emit(*av_list[ai])
                                ai += 1
                    while ai < len(av_list):
                        av_emit(*av_list[ai])
                        ai += 1
                    if prev is not None:
                        for hi in range(2):
                            rec = sb.tile([65, 512], F32, tag="rec", bufs=3)
                            recb = sb.tile([65, 512], BF16, tag="recb", bufs=3)
                            # NOTE: reciprocal_approx_fast misbehaves on
                            # partition slices with base != 0 -- full form.
                            nc.vector.reciprocal_approx_fast(rec[:, :], pops[hi][:65, :])
                            nc.vector.tensor_copy(recb[64:65, :], rec[64:65, :])
                            pend_flush.append((2 * php + hi, php, pqg, pops[hi], recb))
                    if cur is not None:
                        dk = 4 * qg
                        for hi in range(2):  # fused causal mask of diag tiles
                            nc.gpsimd.tensor_mul(
                                pts[hi][:, dk:dk + 4, :128],
                                pts[hi][:, dk:dk + 4, :128], tri4[:])
                        prev = (qg, hp, pts)
                    else:
                        prev = None
                norm_flush(pend_flush)
                pend_flush = []
            with nc.named_scope("proj"):
                for tt in range(TT):
                    proj_tile(tt)

    nc.compile()
    return nc


_NC = None


def _run(in_maps, trace=False, **kwargs):
    global _NC
    if _NC is None:
        _NC = build()
    return bass_utils.run_bass_kernel_spmd(
        _NC, in_maps, core_ids=list(range(N_CORES)), trace=trace, **kwargs
    )


def _tile128(a):
    # [H, C] -> [128, FT*C]: row p holds the p-th partition's data per f-tile
    Hh, C = a.shape
    return np.ascontiguousarray(
        a.reshape(FT, 128, C).transpose(1, 0, 2).reshape(128, FT * C))


def make_in_maps(x, W_attn, b_attn, W_proj, b_proj):
    import ml_dtypes
    bf = ml_dtypes.bfloat16
    f8 = ml_dtypes.float8_e4m3
    x = np.asarray(x, dtype=np.float32)
    W_attn = np.asarray(W_attn, dtype=np.float32)
    b_attn = np.ascontiguousarray(np.asarray(b_attn, dtype=np.float32))
    b_proj = np.asarray(b_proj, dtype=np.float32)
    waqkt = _tile128(W_attn[:, :2 * H].astype(bf))
    wa8t = _tile128((W_attn[:, :2 * H] * WS).astype(f8))
    wa8t = np.ascontiguousarray(
        wa8t.reshape(128, FT, WCH, WCW).transpose(0, 2, 1, 3).reshape(128, -1))
    wavt = _tile128(W_attn[:, 2 * H:].astype(bf))
    wprt = _tile128(np.asarray(W_proj, dtype=np.float32).astype(bf))
    vbias = np.ascontiguousarray(
        np.broadcast_to(b_attn[2 * H:].astype(bf), (128, H)))
    pbias = np.ascontiguousarray(np.broadcast_to(b_proj.astype(bf), (128, H)))
    maps = []
    for b in range(N_CORES):
        xt = _tile128(x[b].T.astype(bf))
        x8t = _tile128((x[b].T.astype(bf).astype(np.float32) * XS).astype(f8))
        maps.append({
            "xt": xt,
            "x8t": x8t,
            "waqkt": waqkt,
            "wa8t": wa8t,
            "wavt": wavt,
            "wprt": wprt,
            "b_attn": b_attn,
            "vbias": vbias,
            "pbias": pbias,
        })
    return maps


def kernel(x, W_attn, b_attn, W_proj, b_proj):
    in_maps = make_in_maps(x, W_attn, b_attn, W_proj, b_proj)
    res = _run(in_maps, trace=False)
    return np.stack([res.results[b]["y"] for b in range(N_CORES)]).astype(np.float32)


# revision 38
# speedup vs baseline: 1.1112x; 1.0084x over previous
"""Multi-head causal self-attention (GPT-style block) on 8 Trainium2 NeuronCores.

Strategy: data-parallel over batch (B=8 -> 1 batch element per core), weights
replicated. Per-core dataflow keeps everything "transposed" so no operand ever
needs an on-chip transpose:

  xT [H,T] bf16 and x8T [H,T] fp8e4 (x16) shipped pre-transposed from host
  q/k projections: fp8 DoubleRow matmuls (W q/k cols pre-scaled x512 fp8,
               K=256 per instruction), evac'd to bf16 by DVE/ACT + bias add
  v    [T,n] = xT-stationary bf16 matmuls over W_attn v-cols
  scores^T [k,q] = bf16 K=64 matmuls on head-parity partition halves
               (no zero padding; tile_position rides partition base 0/64)
  P^T = exp(0.125*scores^T) via ACT, causal diag tiles masked on GpSimd
  out^T[d,q] & softmax denom = [v_h|ones].T @ P^T (ones col -> denom row)
  normalize via PE-broadcast (f32r) of 1/denom, DVE multiply into oT
  y [T,H] = oT-stationary matmuls over W_proj + bias, DVE evac, bf16 out
All remaining matmul compute in bf16 with fp32 PSUM accumulation.
"""

import numpy as np

import concourse.bass as bass
import concourse.mybir as mybir
import concourse.tile as tile
from concourse import bacc, bass_utils
from concourse.masks import make_upper_triangular

F32 = mybir.dt.float32
F32R = mybir.dt.float32r
BF16 = mybir.dt.bfloat16
F8 = mybir.dt.float8e4

T = 1024   # tokens per batch element
H = 768    # hidden
NH = 12    # heads
HS = 64    # head size
TT = T // 128   # token tiles (8)
FT = H // 128   # feature tiles (6)
N_CORES = 8

QK_FP8 = True                   # q/k projection matmuls in fp8 DoubleRow
XS = 16.0                       # fp8 scale for x
WS = 512.0                      # fp8 scale for W_attn q/k columns
EXP_SCALE = 0.125               # folded into the ACT exp
ADD = mybir.AluOpType.add
MULT = mybir.AluOpType.mult
DR = mybir.MatmulPerfMode.DoubleRow
VW = NH * (HS + 1) + 64         # v_bf row width: [v|ones] per head + pad
WCH, WCW = 4, 384               # wa8 chunk count / chunk column width


def build():
    nc = bacc.Bacc(None, target_bir_lowering=False)

    # all inputs pre-tiled on host to [128, FT*cols] so each bulk DMA is
    # 128 contiguous per-partition descriptors
    xT_d = nc.dram_tensor("xt", [128, FT * T], BF16, kind="ExternalInput")
    x8T_d = nc.dram_tensor("x8t", [128, FT * T], F8, kind="ExternalInput")
    waqk_d = nc.dram_tensor("waqkt", [128, FT * 2 * H], BF16, kind="ExternalInput")
    wa8_d = nc.dram_tensor("wa8t", [128, FT * 2 * H], F8, kind="ExternalInput")
    wav_d = nc.dram_tensor("wavt", [128, FT * H], BF16, kind="ExternalInput")
    wp_d = nc.dram_tensor("wprt", [128, FT * H], BF16, kind="ExternalInput")
    ba_d = nc.dram_tensor("b_attn", [3 * H], F32, kind="ExternalInput")
    vb_d = nc.dram_tensor("vbias", [128, H], BF16, kind="ExternalInput")
    pb_d = nc.dram_tensor("pbias", [128, H], BF16, kind="ExternalInput")
    y_d = nc.dram_tensor("y", [T, H], BF16, kind="ExternalOutput")

    with tile.TileContext(nc) as tc:
        with (
            tc.tile_pool(name="sb", bufs=1) as sb,
            tc.tile_pool(name="ps", bufs=1, space="PSUM") as ps,
        ):
            # ---------------- persistent SBUF tensors ----------------
            if QK_FP8:
                wat8 = sb.tile([128, WCH, FT, WCW], F8, tag="wat8")
                xT8 = sb.tile([128, FT, T], F8, tag="xT8")
            else:
                watqk = sb.tile([128, FT, 2 * H], BF16, tag="watqk")
            watv = sb.tile([128, FT, H], BF16, tag="watv")
            wpr = sb.tile([128, FT, H], BF16, tag="wpr")
            xT = sb.tile([128, FT, T], BF16, tag="xT")
            kT = sb.tile([128, FT, T], BF16, tag="kT")   # pair-index major
            qT = sb.tile([128, FT, T], BF16, tag="qT")   # pair-index major
            v_bf = sb.tile([128, TT, VW], BF16, tag="v_bf")  # [v|1] per head
            oT = sb.tile([128, FT, T], BF16, tag="oT")
            bcols = sb.tile([128, 12], F32, tag="bcols")
            vbias = sb.tile([128, H], BF16, tag="vbias")
            pbias = sb.tile([128, H], BF16, tag="pbias")
            onesd = sb.tile([65, 128], BF16, tag="onesd")    # row 64 ones
            tri4 = sb.tile([128, 4, 128], BF16, tag="tri4")  # 4x upper-tri (p<=f)

            # ------- DMA loads (pre-tiled: 128 descriptors per DMA) ---------
            def flat(tile_ap):
                return tile_ap.rearrange("p a b -> p (a b)")

            # q/k matmul operands split across both hwdge queues (concurrent
            # transfers) and chunked so the first QKV matmuls start while the
            # rest is still in flight; everything else follows.  wa8t is
            # shipped chunk-major [128, NC, FT, CW] so chunks stay contiguous.
            if QK_FP8:
                for fp in range(3):
                    nc.scalar.dma_start(
                        flat(xT8[:, 2 * fp:2 * fp + 2, :]),
                        x8T_d[:, fp * 2048:(fp + 1) * 2048])
                for c in range(WCH):
                    nc.sync.dma_start(
                        wat8[:, c, :, :].rearrange("p a b -> p (a b)"),
                        wa8_d[:, c * FT * WCW:(c + 1) * FT * WCW])
            else:
                nc.sync.dma_start(flat(watqk[:]), waqk_d[:, :])
            nc.scalar.dma_start(bcols[:], ba_d[: 12 * 128].rearrange("(t p) -> p t", p=128))
            nc.scalar.dma_start(flat(watv[:]), wav_d[:, :])
            nc.sync.dma_start(flat(xT[:]), xT_d[:, :])
            nc.scalar.dma_start(vbias[:], vb_d[:, :])
            nc.sync.dma_start(wpr[:].rearrange("p a b -> p (a b)"), wp_d[:, :])
            nc.scalar.dma_start(pbias[:], pb_d[:, :])

            # ---------------- constants ----------------
            for r in range(4):
                make_upper_triangular(nc, tri4[:, r, :], val=1.0, diag=True)
            nc.gpsimd.memset(onesd[64:65, :], 1.0)
            nc.gpsimd.memset(v_bf[:, :, NH * (HS + 1):], 0.0)           # tail pad
            nc.gpsimd.memset(v_bf[:, :, HS:NH * (HS + 1):HS + 1], 1.0)  # ones cols

            # ---------------- QKV projection ----------------
            def qk_tile(nt):
                # q^T / k^T : n on partitions; evac'd to fp8 (x QS) on DVE
                pqs = [
                    ps.tile([128, 512], F32, tag="op", bufs=3, name="pq0"),
                    ps.tile([128, 512], F32, tag="op", bufs=3, name="pq1"),
                ]
                if QK_FP8:
                    c, co = divmod(nt * 128, WCW)
                    for fp in range(FT // 2):
                        for tg in range(2):
                            nc.tensor.matmul(
                                pqs[tg][:],
                                wat8[:, c, 2 * fp:2 * fp + 2, co:co + 128],
                                xT8[:, 2 * fp:2 * fp + 2, tg * 512:(tg + 1) * 512],
                                start=(fp == 0),
                                stop=(fp == FT // 2 - 1),
                                perf_mode=DR,
                            )
                else:
                    for ft in range(FT):
                        for tg in range(2):
                            nc.tensor.matmul(
                                pqs[tg][:],
                                watqk[:, ft, nt * 128:(nt + 1) * 128],
                                xT[:, ft, tg * 512:(tg + 1) * 512],
                                start=(ft == 0),
                                stop=(ft == FT - 1),
                            )
                # evac: out = QS*(x@W + b); psum is XS*WS*(x@W) in fp8 mode.
                # Split across DVE (tensor_scalar) and ACT (activation) so
                # neither engine bottlenecks the QKV phase.
                ident_fn = mybir.ActivationFunctionType.Identity
                evac_scale = 1.0 / (XS * WS) if QK_FP8 else 1.0
                for tg in range(2):
                    sl = slice(tg * 512, (tg + 1) * 512)
                    # q/k pair layout: even head rows 0-63, odd rows 64-127
                    dstt = qT if nt < 6 else kT
                    dst = dstt[:, nt % 6, sl]
                    if tg == 0:
                        nc.vector.tensor_scalar(
                            dst, pqs[tg][:],
                            evac_scale, bcols[:, nt:nt + 1], MULT, ADD)
                    else:
                        nc.scalar.activation(
                            dst, pqs[tg][:], ident_fn,
                            bias=bcols[:, nt:nt + 1], scale=evac_scale)

            def v_tile(tt):
                # v : [token, n] natural, bias added via K=1 ones-matmul
                pvs = [
                    ps.tile([128, 512], F32, tag="op", bufs=3, name="pv0"),
                    ps.tile([128, 512], F32, tag="op", bufs=3, name="pv1"),
                ]
                for ft in range(FT):
                    for ng in range(2):
                        w = 512 if ng == 0 else 256
                        nc.tensor.matmul(
                            pvs[ng][:, :w],
                            xT[:, ft, tt * 128:(tt + 1) * 128],
                            watv[:, ft, ng * 512:ng * 512 + w],
                            start=(ft == 0),
                            stop=(ft == FT - 1),
                        )
                v3 = v_bf[:, tt, :NH * (HS + 1)].rearrange("p (h c) -> p h c", c=HS + 1)
                for ng in range(2):
                    w = 512 if ng == 0 else 256
                    hlo = ng * 8
                    hhi = 8 if ng == 0 else 12
                    nc.vector.tensor_add(
                        v3[:, hlo:hhi, :HS],
                        pvs[ng][:, :w].rearrange("p (h d) -> p h d", d=HS),
                        vbias[:, ng * 512:ng * 512 + w].rearrange("p (h d) -> p h d", d=HS),
                    )

            # ---------------- attention + interleaved projection ------------
            # Normalize chain of each group deferred by one group so the PE
            # never head-of-line blocks on the DVE reciprocal.
            def norm_flush(pending):
                for h_, hp_, qg_, op_, recb_ in pending:
                    bp = ps.tile([128, 512], F32, tag="bc", bufs=1, name="bp")
                    nc.tensor.matmul(
                        bp[:], onesd[64:65, :], recb_[64:65, :],
                        start=True, stop=True,
                    )
                    bpb = sb.tile([64, 512], BF16, tag="bpb", bufs=2, name="bpb")
                    nc.vector.tensor_copy(bpb[:], bp[:64, :])
                    dst = slice(512 * qg_, 512 * (qg_ + 1))
                    if h_ % 2 == 0:
                        nc.vector.tensor_mul(
                            oT[:64, hp_, dst], op_[:64, :], bpb[:])
                    else:
                        sc = sb.tile([64, 512], BF16, tag="sc", bufs=3, name="sc")
                        nc.vector.tensor_mul(sc[:], op_[:64, :], bpb[:])
                        nc.sync.dma_start(oT[64:128, hp_, dst], sc[:])

            def emit_scores_pair(qg, hp, pts, kt0, kt1):
                offs, ws = [], []
                for kt in (kt0, kt1):
                    q_off = max(128 * kt, 512 * qg)
                    offs.append(q_off)
                    ws.append(512 * (qg + 1) - q_off)
                vw = 512 + ws[1]  # exp span: slot0 prefix + slot1 valid
                sps2 = [
                    ps.tile([128, 1024], F32, tag="sp0", bufs=1, name="spA"),
                    ps.tile([128, 1024], F32, tag="sp1", bufs=1, name="spB"),
                ]
                for j, kt in enumerate((kt0, kt1)):
                    for hi in range(2):
                        rows = slice(64 * hi, 64 * hi + 64)
                        nc.tensor.matmul(
                            sps2[hi][:, j * 512:j * 512 + ws[j]],
                            kT[rows, hp, kt * 128:(kt + 1) * 128],
                            qT[rows, hp, offs[j]:offs[j] + ws[j]],
                            start=True,
                            stop=True,
                        )
                for hi in range(2):
                    dst = pts[hi][:, kt0:kt0 + 2, :].rearrange("p a b -> p (a b)")
                    nc.scalar.activation(
                        dst[:, :vw], sps2[hi][:, :vw],
                        mybir.ActivationFunctionType.Exp, scale=EXP_SCALE,
                    )

            def proj_tile(tt):
                pys = [
                    ps.tile([128, 512], F32, tag="op", bufs=3, name="py0"),
                    ps.tile([128, 512], F32, tag="op", bufs=3, name="py1"),
                ]
                for ft in range(FT):
                    for ng in range(2):
                        w = 512 if ng == 0 else 256
                        nc.tensor.matmul(
                            pys[ng][:, :w],
                            oT[:, ft, tt * 128:(tt + 1) * 128],
                            wpr[:, ft, ng * 512:ng * 512 + w],
                            start=(ft == 0),
                            stop=(ft == FT - 1),
                        )
                ysb = sb.tile([128, H], BF16, tag="ysb", bufs=2)
                for ng in range(2):
                    w = 512 if ng == 0 else 256
                    nc.vector.tensor_add(
                        ysb[:, ng * 512:ng * 512 + w], pys[ng][:, :w],
                        pbias[:, ng * 512:ng * 512 + w])
                nc.sync.dma_start(y_d[tt * 128:(tt + 1) * 128, :], ysb[:])

            with nc.named_scope("qkv"):
                for nt in range(NH):
                    qk_tile(nt)
                for tt in range(TT):
                    v_tile(tt)

            # Software pipeline over head-pair groups: at step n the PE
            # emits scores(n) interleaved with AV(n-1); normalize of (n-1)
            # flushes at step n+1.  PE never queue-blocks on exp/recip.
            groups = [(0, hp) for hp in range(6)] + [(1, hp) for hp in range(6)]
            pend_flush = []
            prev = None  # (qg, hp, pts) of previous group, AV not yet emitted
            with nc.named_scope("attn"):
                for step in range(len(groups) + 1):
                    cur = groups[step] if step < len(groups) else None
                    norm_flush(pend_flush)
                    pend_flush = []
                    av_list = []
                    if prev is not None:
                        pqg, php, ppts = prev
                        pkts = list(range(4 * pqg + 4))
                        pops = [
                            ps.tile([128, 512], F32, tag="op", bufs=3, name=f"av{hi}")
                            for hi in range(2)
                        ]

                        def av_emit(hi, kt, jk, _pqg=pqg, _php=php, _ppts=ppts,
                                    _pops=pops, _nk=len(pkts)):
                            q_off = max(128 * kt, 512 * _pqg)
                            w = 512 * (_pqg + 1) - q_off
                            off = q_off - 512 * _pqg
                            h = 2 * _php + hi
                            nc.tensor.matmul(
                                _pops[hi][:, off:off + w],
                                v_bf[:, kt, 65 * h:65 * h + 128],
                                _ppts[hi][:, kt, :w],
                                start=(jk == 0),
                                stop=(jk == _nk - 1),
                            )
                        av_list = [(hi, kt, jk)
                                   for hi in range(2)
                                   for jk, kt in enumerate(pkts)]
                    if cur is not None:
                        qg, hp = cur
                        kts = list(range(4 * qg + 4))
                        pts = [
                            sb.tile([128, 8, 512], BF16, tag=f"pT{hi}", bufs=3, name=f"pt{hi}")
                            for hi in range(2)
                        ]
                        pairs = [(kts[i], kts[i + 1]) for i in range(0, len(kts), 2)]
                    else:
                        pairs = []
                    def emit_recip(hi):
                        rec = sb.tile([65, 512], F32, tag="rec", bufs=3)
                        recb = sb.tile([65, 512], BF16, tag="recb", bufs=3)
                        # NOTE: reciprocal_approx_fast misbehaves on
                        # partition slices with base != 0 -- full form.
                        nc.vector.reciprocal_approx_fast(rec[:, :], pops[hi][:65, :])
                        nc.vector.tensor_copy(recb[64:65, :], rec[64:65, :])
                        pend_flush.append((2 * php + hi, php, pqg, pops[hi], recb))

                    nchunks = max(len(pairs), 1)
                    per = -(-len(av_list) // nchunks) if av_list else 0
                    ai = 0
                    nk = len(pkts) if prev is not None else 0

                    def av_step():
                        # emit each head's reciprocal as soon as its AV
                        # accumulation chain ends, so the next step's flush
                        # never waits on a DVE backlog
                        nonlocal ai
                        av_emit(*av_list[ai])
                        ai += 1
                        if ai == nk:
                            emit_recip(0)
                        elif ai == 2 * nk:
                            emit_recip(1)

                    for pi in range(nchunks):
                        if pi < len(pairs):
                            emit_scores_pair(qg, hp, pts, *pairs[pi])
                        for _ in range(per):
                            if ai < len(av_list):
                                av_step()
                    while ai < len(av_list):
                        av_step()
                    if cur is not None:
                        dk = 4 * qg
                        for hi in range(2):  # fused causal mask of diag tiles
                            nc.gpsimd.tensor_mul(
                                pts[hi][:, dk:dk + 4, :128],
                                pts[hi][:, dk:dk + 4, :128], tri4[:])
                        prev = (qg, hp, pts)
                    else:
                        prev = None
                norm_flush(pend_flush)
                pend_flush = []
            with nc.named_scope("proj"):
                for tt in range(TT):
                    proj_tile(tt)

    nc.compile()
    return nc


_NC = None


def _run(in_maps, trace=False, **kwargs):
    global _NC
    if _NC is None:
        _NC = build()
    return bass_utils.run_bass_kernel_spmd(
        _NC, in_maps, core_ids=list(range(N_CORES)), trace=trace, **kwargs
    )


def _tile128(a):
    # [H, C] -> [128, FT*C]: row p holds the p-th partition's data per f-tile
    Hh, C = a.shape
    return np.ascontiguousarray(
        a.reshape(FT, 128, C).transpose(1, 0, 2).reshape(128, FT * C))


def make_in_maps(x, W_attn, b_attn, W_proj, b_proj):
    import ml_dtypes
    bf = ml_dtypes.bfloat16
    f8 = ml_dtypes.float8_e4m3
    x = np.asarray(x, dtype=np.float32)
    W_attn = np.asarray(W_attn, dtype=np.float32)
    b_attn = np.ascontiguousarray(np.asarray(b_attn, dtype=np.float32))
    b_proj = np.asarray(b_proj, dtype=np.float32)
    waqkt = _tile128(W_attn[:, :2 * H].astype(bf))
    wa8t = _tile128((W_attn[:, :2 * H] * WS).astype(f8))
    wa8t = np.ascontiguousarray(
        wa8t.reshape(128, FT, WCH, WCW).transpose(0, 2, 1, 3).reshape(128, -1))
    wavt = _tile128(W_attn[:, 2 * H:].astype(bf))
    wprt = _tile128(np.asarray(W_proj, dtype=np.float32).astype(bf))
    vbias = np.ascontiguousarray(
        np.broadcast_to(b_attn[2 * H:].astype(bf), (128, H)))
    pbias = np.ascontiguousarray(np.broadcast_to(b_proj.astype(bf), (128, H)))
    maps = []
    for b in range(N_CORES):
        xt = _tile128(x[b].T.astype(bf))
        x8t = _tile128((x[b].T.astype(bf).astype(np.float32) * XS).astype(f8))
        maps.append({
            "xt": xt,
            "x8t": x8t,
            "waqkt": waqkt,
            "wa8t": wa8t,
            "wavt": wavt,
            "wprt": wprt,
            "b_attn": b_attn,
            "vbias": vbias,
            "pbias": pbias,
        })
    return maps


def kernel(x, W_attn, b_attn, W_proj, b_proj):
    in_maps = make_in_maps(x, W_attn, b_attn, W_proj, b_proj)
    res = _run(in_maps, trace=False)
    return np.stack([res.results[b]["y"] for b in range(N_CORES)]).astype(np.float32)


# revision 39
# speedup vs baseline: 1.1172x; 1.0054x over previous
"""Multi-head causal self-attention (GPT-style block) on 8 Trainium2 NeuronCores.

Strategy: data-parallel over batch (B=8 -> 1 batch element per core), weights
replicated. Per-core dataflow keeps everything "transposed" so no operand ever
needs an on-chip transpose:

  xT [H,T] bf16 and x8T [H,T] fp8e4 (x16) shipped pre-transposed from host
  q/k projections: fp8 DoubleRow matmuls (W q/k cols pre-scaled x512 fp8,
               K=256 per instruction), evac'd to bf16 by DVE/ACT + bias add
  v    [T,n] = xT-stationary bf16 matmuls over W_attn v-cols
  scores^T [k,q] = bf16 K=64 matmuls on head-parity partition halves
               (no zero padding; tile_position rides partition base 0/64)
  P^T = exp(0.125*scores^T) via ACT, causal diag tiles masked on GpSimd
  out^T[d,q] & softmax denom = [v_h|ones].T @ P^T (ones col -> denom row)
  normalize via PE-broadcast (f32r) of 1/denom, DVE multiply into oT
  y [T,H] = oT-stationary matmuls over W_proj + bias, DVE evac, bf16 out
All remaining matmul compute in bf16 with fp32 PSUM accumulation.
"""

import numpy as np

import concourse.bass as bass
import concourse.mybir as mybir
import concourse.tile as tile
from concourse import bacc, bass_utils
from concourse.masks import make_upper_triangular

F32 = mybir.dt.float32
F32R = mybir.dt.float32r
BF16 = mybir.dt.bfloat16
F8 = mybir.dt.float8e4

T = 1024   # tokens per batch element
H = 768    # hidden
NH = 12    # heads
HS = 64    # head size
TT = T // 128   # token tiles (8)
FT = H // 128   # feature tiles (6)
N_CORES = 8

QK_FP8 = True                   # q/k projection matmuls in fp8 DoubleRow
XS = 16.0                       # fp8 scale for x
WS = 512.0                      # fp8 scale for W_attn q/k columns
EXP_SCALE = 0.125               # folded into the ACT exp
ADD = mybir.AluOpType.add
MULT = mybir.AluOpType.mult
DR = mybir.MatmulPerfMode.DoubleRow
VW = NH * (HS + 1) + 64         # v_bf row width: [v|ones] per head + pad
WCH, WCW = 4, 384               # wa8 chunk count / chunk column width


def build():
    nc = bacc.Bacc(None, target_bir_lowering=False)

    # all inputs pre-tiled on host to [128, FT*cols] so each bulk DMA is
    # 128 contiguous per-partition descriptors
    xT_d = nc.dram_tensor("xt", [128, FT * T], BF16, kind="ExternalInput")
    x8T_d = nc.dram_tensor("x8t", [128, FT * T], F8, kind="ExternalInput")
    waqk_d = nc.dram_tensor("waqkt", [128, FT * 2 * H], BF16, kind="ExternalInput")
    wa8_d = nc.dram_tensor("wa8t", [128, FT * 2 * H], F8, kind="ExternalInput")
    wav_d = nc.dram_tensor("wavt", [128, FT * H], BF16, kind="ExternalInput")
    wp_d = nc.dram_tensor("wprt", [128, FT * H], BF16, kind="ExternalInput")
    ba_d = nc.dram_tensor("b_attn", [3 * H], F32, kind="ExternalInput")
    vb_d = nc.dram_tensor("vbias", [128, H], BF16, kind="ExternalInput")
    pb_d = nc.dram_tensor("pbias", [128, H], BF16, kind="ExternalInput")
    y_d = nc.dram_tensor("y", [T, H], BF16, kind="ExternalOutput")

    with tile.TileContext(nc) as tc:
        with (
            tc.tile_pool(name="sb", bufs=1) as sb,
            tc.tile_pool(name="ps", bufs=1, space="PSUM") as ps,
        ):
            # ---------------- persistent SBUF tensors ----------------
            if QK_FP8:
                wat8 = sb.tile([128, WCH, FT, WCW], F8, tag="wat8")
                xT8 = sb.tile([128, FT, T], F8, tag="xT8")
            else:
                watqk = sb.tile([128, FT, 2 * H], BF16, tag="watqk")
            watv = sb.tile([128, FT, H], BF16, tag="watv")
            wpr = sb.tile([128, FT, H], BF16, tag="wpr")
            xT = sb.tile([128, FT, T], BF16, tag="xT")
            kT = sb.tile([128, FT, T], BF16, tag="kT")   # pair-index major
            qT = sb.tile([128, FT, T], BF16, tag="qT")   # pair-index major
            v_bf = sb.tile([128, TT, VW], BF16, tag="v_bf")  # [v|1] per head
            oT = sb.tile([128, FT, T], BF16, tag="oT")
            bcols = sb.tile([128, 12], F32, tag="bcols")
            vbias = sb.tile([128, H], BF16, tag="vbias")
            pbias = sb.tile([128, H], BF16, tag="pbias")
            onesd = sb.tile([65, 128], BF16, tag="onesd")    # row 64 ones
            tri4 = sb.tile([128, 4, 128], BF16, tag="tri4")  # 4x upper-tri (p<=f)

            # ------- DMA loads (pre-tiled: 128 descriptors per DMA) ---------
            def flat(tile_ap):
                return tile_ap.rearrange("p a b -> p (a b)")

            # q/k matmul operands split across both hwdge queues (concurrent
            # transfers) and chunked so the first QKV matmuls start while the
            # rest is still in flight; everything else follows.  wa8t is
            # shipped chunk-major [128, NC, FT, CW] so chunks stay contiguous.
            if QK_FP8:
                for fp in range(3):
                    nc.scalar.dma_start(
                        flat(xT8[:, 2 * fp:2 * fp + 2, :]),
                        x8T_d[:, fp * 2048:(fp + 1) * 2048])
                for c in range(WCH):
                    nc.sync.dma_start(
                        wat8[:, c, :, :].rearrange("p a b -> p (a b)"),
                        wa8_d[:, c * FT * WCW:(c + 1) * FT * WCW])
            else:
                nc.sync.dma_start(flat(watqk[:]), waqk_d[:, :])
            nc.scalar.dma_start(bcols[:], ba_d[: 12 * 128].rearrange("(t p) -> p t", p=128))
            nc.scalar.dma_start(flat(watv[:]), wav_d[:, :])
            # xT rides the third (gpsimd swdge) queue so its transfer overlaps
            # the wat8/x8T loads and is resident before the first v matmul
            nc.gpsimd.dma_start(flat(xT[:]), xT_d[:, :])
            nc.scalar.dma_start(vbias[:], vb_d[:, :])
            nc.gpsimd.dma_start(wpr[:].rearrange("p a b -> p (a b)"), wp_d[:, :])
            nc.scalar.dma_start(pbias[:], pb_d[:, :])

            # ---------------- constants ----------------
            for r in range(4):
                make_upper_triangular(nc, tri4[:, r, :], val=1.0, diag=True)
            nc.gpsimd.memset(onesd[64:65, :], 1.0)
            nc.gpsimd.memset(v_bf[:, :, NH * (HS + 1):], 0.0)           # tail pad
            nc.gpsimd.memset(v_bf[:, :, HS:NH * (HS + 1):HS + 1], 1.0)  # ones cols

            # ---------------- QKV projection ----------------
            def qk_tile(nt):
                # q^T / k^T : n on partitions; evac'd to fp8 (x QS) on DVE
                pqs = [
                    ps.tile([128, 512], F32, tag="op", bufs=3, name="pq0"),
                    ps.tile([128, 512], F32, tag="op", bufs=3, name="pq1"),
                ]
                if QK_FP8:
                    c, co = divmod(nt * 128, WCW)
                    for fp in range(FT // 2):
                        for tg in range(2):
                            nc.tensor.matmul(
                                pqs[tg][:],
                                wat8[:, c, 2 * fp:2 * fp + 2, co:co + 128],
                                xT8[:, 2 * fp:2 * fp + 2, tg * 512:(tg + 1) * 512],
                                start=(fp == 0),
                                stop=(fp == FT // 2 - 1),
                                perf_mode=DR,
                            )
                else:
                    for ft in range(FT):
                        for tg in range(2):
                            nc.tensor.matmul(
                                pqs[tg][:],
                                watqk[:, ft, nt * 128:(nt + 1) * 128],
                                xT[:, ft, tg * 512:(tg + 1) * 512],
                                start=(ft == 0),
                                stop=(ft == FT - 1),
                            )
                # evac: out = QS*(x@W + b); psum is XS*WS*(x@W) in fp8 mode.
                # Split across DVE (tensor_scalar) and ACT (activation) so
                # neither engine bottlenecks the QKV phase.
                ident_fn = mybir.ActivationFunctionType.Identity
                evac_scale = 1.0 / (XS * WS) if QK_FP8 else 1.0
                for tg in range(2):
                    sl = slice(tg * 512, (tg + 1) * 512)
                    # q/k pair layout: even head rows 0-63, odd rows 64-127
                    dstt = qT if nt < 6 else kT
                    dst = dstt[:, nt % 6, sl]
                    if tg == 0:
                        nc.vector.tensor_scalar(
                            dst, pqs[tg][:],
                            evac_scale, bcols[:, nt:nt + 1], MULT, ADD)
                    else:
                        nc.scalar.activation(
                            dst, pqs[tg][:], ident_fn,
                            bias=bcols[:, nt:nt + 1], scale=evac_scale)

            def v_tile(tt):
                # v : [token, n] natural, bias added via K=1 ones-matmul
                pvs = [
                    ps.tile([128, 512], F32, tag="op", bufs=3, name="pv0"),
                    ps.tile([128, 512], F32, tag="op", bufs=3, name="pv1"),
                ]
                for ft in range(FT):
                    for ng in range(2):
                        w = 512 if ng == 0 else 256
                        nc.tensor.matmul(
                            pvs[ng][:, :w],
                            xT[:, ft, tt * 128:(tt + 1) * 128],
                            watv[:, ft, ng * 512:ng * 512 + w],
                            start=(ft == 0),
                            stop=(ft == FT - 1),
                        )
                v3 = v_bf[:, tt, :NH * (HS + 1)].rearrange("p (h c) -> p h c", c=HS + 1)
                for ng in range(2):
                    w = 512 if ng == 0 else 256
                    hlo = ng * 8
                    hhi = 8 if ng == 0 else 12
                    nc.vector.tensor_add(
                        v3[:, hlo:hhi, :HS],
                        pvs[ng][:, :w].rearrange("p (h d) -> p h d", d=HS),
                        vbias[:, ng * 512:ng * 512 + w].rearrange("p (h d) -> p h d", d=HS),
                    )

            # ---------------- attention + interleaved projection ------------
            # Normalize chain of each group deferred by one group so the PE
            # never head-of-line blocks on the DVE reciprocal.
            def norm_flush(pending):
                for h_, hp_, qg_, op_, recb_ in pending:
                    bp = ps.tile([128, 512], F32, tag="bc", bufs=1, name="bp")
                    nc.tensor.matmul(
                        bp[:], onesd[64:65, :], recb_[64:65, :],
                        start=True, stop=True,
                    )
                    bpb = sb.tile([64, 512], BF16, tag="bpb", bufs=2, name="bpb")
                    nc.vector.tensor_copy(bpb[:], bp[:64, :])
                    dst = slice(512 * qg_, 512 * (qg_ + 1))
                    if h_ % 2 == 0:
                        nc.vector.tensor_mul(
                            oT[:64, hp_, dst], op_[:64, :], bpb[:])
                    else:
                        sc = sb.tile([64, 512], BF16, tag="sc", bufs=3, name="sc")
                        nc.vector.tensor_mul(sc[:], op_[:64, :], bpb[:])
                        nc.sync.dma_start(oT[64:128, hp_, dst], sc[:])

            def emit_scores_pair(qg, hp, pts, kt0, kt1):
                offs, ws = [], []
                for kt in (kt0, kt1):
                    q_off = max(128 * kt, 512 * qg)
                    offs.append(q_off)
                    ws.append(512 * (qg + 1) - q_off)
                vw = 512 + ws[1]  # exp span: slot0 prefix + slot1 valid
                sps2 = [
                    ps.tile([128, 1024], F32, tag="sp0", bufs=1, name="spA"),
                    ps.tile([128, 1024], F32, tag="sp1", bufs=1, name="spB"),
                ]
                for j, kt in enumerate((kt0, kt1)):
                    for hi in range(2):
                        rows = slice(64 * hi, 64 * hi + 64)
                        nc.tensor.matmul(
                            sps2[hi][:, j * 512:j * 512 + ws[j]],
                            kT[rows, hp, kt * 128:(kt + 1) * 128],
                            qT[rows, hp, offs[j]:offs[j] + ws[j]],
                            start=True,
                            stop=True,
                        )
                for hi in range(2):
                    dst = pts[hi][:, kt0:kt0 + 2, :].rearrange("p a b -> p (a b)")
                    nc.scalar.activation(
                        dst[:, :vw], sps2[hi][:, :vw],
                        mybir.ActivationFunctionType.Exp, scale=EXP_SCALE,
                    )

            def proj_tile(tt):
                pys = [
                    ps.tile([128, 512], F32, tag="op", bufs=3, name="py0"),
                    ps.tile([128, 512], F32, tag="op", bufs=3, name="py1"),
                ]
                for ft in range(FT):
                    for ng in range(2):
                        w = 512 if ng == 0 else 256
                        nc.tensor.matmul(
                            pys[ng][:, :w],
                            oT[:, ft, tt * 128:(tt + 1) * 128],
                            wpr[:, ft, ng * 512:ng * 512 + w],
                            start=(ft == 0),
                            stop=(ft == FT - 1),
                        )
                ysb = sb.tile([128, H], BF16, tag="ysb", bufs=2)
                for ng in range(2):
                    w = 512 if ng == 0 else 256
                    nc.vector.tensor_add(
                        ysb[:, ng * 512:ng * 512 + w], pys[ng][:, :w],
                        pbias[:, ng * 512:ng * 512 + w])
                nc.sync.dma_start(y_d[tt * 128:(tt + 1) * 128, :], ysb[:])

            with nc.named_scope("qkv"):
                for nt in range(NH):
                    qk_tile(nt)
                for tt in range(TT):
                    v_tile(tt)

            # Software pipeline over head-pair groups: at step n the PE
            # emits scores(n) interleaved with AV(n-1); normalize of (n-1)
            # flushes at step n+1.  PE never queue-blocks on exp/recip.
            groups = [(0, hp) for hp in range(6)] + [(1, hp) for hp in range(6)]
            pend_flush = []
            prev = None  # (qg, hp, pts) of previous group, AV not yet emitted
            with nc.named_scope("attn"):
                for step in range(len(groups) + 1):
                    cur = groups[step] if step < len(groups) else None
                    norm_flush(pend_flush)
                    pend_flush = []
                    av_list = []
                    if prev is not None:
                        pqg, php, ppts = prev
                        pkts = list(range(4 * pqg + 4))
                        pops = [
                            ps.tile([128, 512], F32, tag="op", bufs=3, name=f"av{hi}")
                            for hi in range(2)
                        ]

                        def av_emit(hi, kt, jk, _pqg=pqg, _php=php, _ppts=ppts,
                                    _pops=pops, _nk=len(pkts)):
                            q_off = max(128 * kt, 512 * _pqg)
                            w = 512 * (_pqg + 1) - q_off
                            off = q_off - 512 * _pqg
                            h = 2 * _php + hi
                            nc.tensor.matmul(
                                _pops[hi][:, off:off + w],
                                v_bf[:, kt, 65 * h:65 * h + 128],
                                _ppts[hi][:, kt, :w],
                                start=(jk == 0),
                                stop=(jk == _nk - 1),
                            )
                        av_list = [(hi, kt, jk)
                                   for hi in range(2)
                                   for jk, kt in enumerate(pkts)]
                    if cur is not None:
                        qg, hp = cur
                        kts = list(range(4 * qg + 4))
                        pts = [
                            sb.tile([128, 8, 512], BF16, tag=f"pT{hi}", bufs=3, name=f"pt{hi}")
                            for hi in range(2)
                        ]
                        pairs = [(kts[i], kts[i + 1]) for i in range(0, len(kts), 2)]
                    else:
                        pairs = []
                    def emit_recip(hi):
                        rec = sb.tile([65, 512], F32, tag="rec", bufs=3)
                        recb = sb.tile([65, 512], BF16, tag="recb", bufs=3)
                        # NOTE: reciprocal_approx_fast misbehaves on
                        # partition slices with base != 0 -- full form.
                        nc.vector.reciprocal_approx_fast(rec[:, :], pops[hi][:65, :])
                        nc.vector.tensor_copy(recb[64:65, :], rec[64:65, :])
                        pend_flush.append((2 * php + hi, php, pqg, pops[hi], recb))

                    nchunks = max(len(pairs), 1)
                    per = -(-len(av_list) // nchunks) if av_list else 0
                    ai = 0
                    nk = len(pkts) if prev is not None else 0

                    def av_step():
                        # emit each head's reciprocal as soon as its AV
                        # accumulation chain ends, so the next step's flush
                        # never waits on a DVE backlog
                        nonlocal ai
                        av_emit(*av_list[ai])
                        ai += 1
                        if ai == nk:
                            emit_recip(0)
                        elif ai == 2 * nk:
                            emit_recip(1)

                    for pi in range(nchunks):
                        if pi < len(pairs):
                            emit_scores_pair(qg, hp, pts, *pairs[pi])
                        for _ in range(per):
                            if ai < len(av_list):
                                av_step()
                    while ai < len(av_list):
                        av_step()
                    if cur is not None:
                        dk = 4 * qg
                        for hi in range(2):  # fused causal mask of diag tiles
                            nc.gpsimd.tensor_mul(
                                pts[hi][:, dk:dk + 4, :128],
                                pts[hi][:, dk:dk + 4, :128], tri4[:])
                        prev = (qg, hp, pts)
                    else:
                        prev = None
                norm_flush(pend_flush)
                pend_flush = []
            with nc.named_scope("proj"):
                for tt in range(TT):
                    proj_tile(tt)

    nc.compile()
    return nc


_NC = None


def _run(in_maps, trace=False, **kwargs):
    global _NC
    if _NC is None:
        _NC = build()
    return bass_utils.run_bass_kernel_spmd(
        _NC, in_maps, core_ids=list(range(N_CORES)), trace=trace, **kwargs
    )


def _tile128(a):
    # [H, C] -> [128, FT*C]: row p holds the p-th partition's data per f-tile
    Hh, C = a.shape
    return np.ascontiguousarray(
        a.reshape(FT, 128, C).transpose(1, 0, 2).reshape(128, FT * C))


def make_in_maps(x, W_attn, b_attn, W_proj, b_proj):
    import ml_dtypes
    bf = ml_dtypes.bfloat16
    f8 = ml_dtypes.float8_e4m3
    x = np.asarray(x, dtype=np.float32)
    W_attn = np.asarray(W_attn, dtype=np.float32)
    b_attn = np.ascontiguousarray(np.asarray(b_attn, dtype=np.float32))
    b_proj = np.asarray(b_proj, dtype=np.float32)
    waqkt = _tile128(W_attn[:, :2 * H].astype(bf))
    wa8t = _tile128((W_attn[:, :2 * H] * WS).astype(f8))
    wa8t = np.ascontiguousarray(
        wa8t.reshape(128, FT, WCH, WCW).transpose(0, 2, 1, 3).reshape(128, -1))
    wavt = _tile128(W_attn[:, 2 * H:].astype(bf))
    wprt = _tile128(np.asarray(W_proj, dtype=np.float32).astype(bf))
    vbias = np.ascontiguousarray(
        np.broadcast_to(b_attn[2 * H:].astype(bf), (128, H)))
    pbias = np.ascontiguousarray(np.broadcast_to(b_proj.astype(bf), (128, H)))
    maps = []
    for b in range(N_CORES):
        xt = _tile128(x[b].T.astype(bf))
        x8t = _tile128((x[b].T.astype(bf).astype(np.float32) * XS).astype(f8))
        maps.append({
            "xt": xt,
            "x8t": x8t,
            "waqkt": waqkt,
            "wa8t": wa8t,
            "wavt": wavt,
            "wprt": wprt,
            "b_attn": b_attn,
            "vbias": vbias,
            "pbias": pbias,
        })
    return maps


def kernel(x, W_attn, b_attn, W_proj, b_proj):
    in_maps = make_in_maps(x, W_attn, b_attn, W_proj, b_proj)
    res = _run(in_maps, trace=False)
    return np.stack([res.results[b]["y"] for b in range(N_CORES)]).astype(np.float32)


# revision 40
# speedup vs baseline: 1.1235x; 1.0057x over previous
"""Multi-head causal self-attention (GPT-style block) on 8 Trainium2 NeuronCores.

Strategy: data-parallel over batch (B=8 -> 1 batch element per core), weights
replicated. Per-core dataflow keeps everything "transposed" so no operand ever
needs an on-chip transpose:

  xT [H,T] bf16 and x8T [H,T] fp8e4 (x16) shipped pre-transposed from host
  q/k projections: fp8 DoubleRow matmuls (W q/k cols pre-scaled x512 fp8,
               K=256 per instruction), evac'd to bf16 by DVE/ACT + bias add
  v    [T,n] = xT-stationary bf16 matmuls over W_attn v-cols
  scores^T [k,q] = bf16 K=64 matmuls on head-parity partition halves
               (no zero padding; tile_position rides partition base 0/64)
  P^T = exp(0.125*scores^T) via ACT, causal diag tiles masked on GpSimd
  out^T[d,q] & softmax denom = [v_h|ones].T @ P^T (ones col -> denom row)
  normalize via PE-broadcast (f32r) of 1/denom, DVE multiply into oT
  y [T,H] = oT-stationary matmuls over W_proj + bias, DVE evac, bf16 out
All remaining matmul compute in bf16 with fp32 PSUM accumulation.
"""

import numpy as np

import concourse.bass as bass
import concourse.mybir as mybir
import concourse.tile as tile
from concourse import bacc, bass_utils
from concourse.masks import make_upper_triangular

F32 = mybir.dt.float32
F32R = mybir.dt.float32r
BF16 = mybir.dt.bfloat16
F8 = mybir.dt.float8e4

T = 1024   # tokens per batch element
H = 768    # hidden
NH = 12    # heads
HS = 64    # head size
TT = T // 128   # token tiles (8)
FT = H // 128   # feature tiles (6)
N_CORES = 8

QK_FP8 = True                   # q/k projection matmuls in fp8 DoubleRow
XS = 16.0                       # fp8 scale for x
WS = 512.0                      # fp8 scale for W_attn q/k columns
EXP_SCALE = 0.125               # folded into the ACT exp
ADD = mybir.AluOpType.add
MULT = mybir.AluOpType.mult
DR = mybir.MatmulPerfMode.DoubleRow
VW = NH * (HS + 1) + 64         # v_bf row width: [v|ones] per head + pad
WCH, WCW = 4, 384               # wa8 chunk count / chunk column width


def build():
    nc = bacc.Bacc(None, target_bir_lowering=False)

    # all inputs pre-tiled on host to [128, FT*cols] so each bulk DMA is
    # 128 contiguous per-partition descriptors
    xT_d = nc.dram_tensor("xt", [128, FT * T], BF16, kind="ExternalInput")
    x8T_d = nc.dram_tensor("x8t", [128, FT * T], F8, kind="ExternalInput")
    waqk_d = nc.dram_tensor("waqkt", [128, FT * 2 * H], BF16, kind="ExternalInput")
    wa8_d = nc.dram_tensor("wa8t", [128, FT * 2 * H], F8, kind="ExternalInput")
    wav_d = nc.dram_tensor("wavt", [128, FT * H], BF16, kind="ExternalInput")
    wp_d = nc.dram_tensor("wprt", [128, FT * H], BF16, kind="ExternalInput")
    ba_d = nc.dram_tensor("b_attn", [3 * H], F32, kind="ExternalInput")
    vb_d = nc.dram_tensor("vbias", [128, H], BF16, kind="ExternalInput")
    pb_d = nc.dram_tensor("pbias", [128, H], BF16, kind="ExternalInput")
    y_d = nc.dram_tensor("y", [T, H], BF16, kind="ExternalOutput")

    with tile.TileContext(nc) as tc:
        with (
            tc.tile_pool(name="sb", bufs=1) as sb,
            tc.tile_pool(name="ps", bufs=1, space="PSUM") as ps,
        ):
            # ---------------- persistent SBUF tensors ----------------
            if QK_FP8:
                wat8 = sb.tile([128, WCH, FT, WCW], F8, tag="wat8")
                xT8 = sb.tile([128, FT, T], F8, tag="xT8")
            else:
                watqk = sb.tile([128, FT, 2 * H], BF16, tag="watqk")
            watv = sb.tile([128, FT, H], BF16, tag="watv")
            wpr = sb.tile([128, FT, H], BF16, tag="wpr")
            xT = sb.tile([128, FT, T], BF16, tag="xT")
            kT = sb.tile([128, FT, T], BF16, tag="kT")   # pair-index major
            qT = sb.tile([128, FT, T], BF16, tag="qT")   # pair-index major
            v_bf = sb.tile([128, TT, VW], BF16, tag="v_bf")  # [v|1] per head
            oT = sb.tile([128, FT, T], BF16, tag="oT")
            bcols = sb.tile([128, 12], F32, tag="bcols")
            vbias = sb.tile([128, H], BF16, tag="vbias")
            pbias = sb.tile([128, H], BF16, tag="pbias")
            onesd = sb.tile([65, 128], BF16, tag="onesd")    # row 64 ones
            tri4 = sb.tile([128, 4, 128], BF16, tag="tri4")  # 4x upper-tri (p<=f)

            # ------- DMA loads (pre-tiled: 128 descriptors per DMA) ---------
            def flat(tile_ap):
                return tile_ap.rearrange("p a b -> p (a b)")

            # q/k matmul operands split across both hwdge queues (concurrent
            # transfers) and chunked so the first QKV matmuls start while the
            # rest is still in flight; everything else follows.  wa8t is
            # shipped chunk-major [128, NC, FT, CW] so chunks stay contiguous.
            if QK_FP8:
                for fp in range(3):
                    nc.scalar.dma_start(
                        flat(xT8[:, 2 * fp:2 * fp + 2, :]),
                        x8T_d[:, fp * 2048:(fp + 1) * 2048])
                for c in range(WCH):
                    nc.sync.dma_start(
                        wat8[:, c, :, :].rearrange("p a b -> p (a b)"),
                        wa8_d[:, c * FT * WCW:(c + 1) * FT * WCW])
            else:
                nc.sync.dma_start(flat(watqk[:]), waqk_d[:, :])
            nc.scalar.dma_start(bcols[:], ba_d[: 12 * 128].rearrange("(t p) -> p t", p=128))
            nc.scalar.dma_start(flat(watv[:]), wav_d[:, :])
            # xT rides the third (gpsimd swdge) queue so its transfer overlaps
            # the wat8/x8T loads and is resident before the first v matmul
            nc.gpsimd.dma_start(flat(xT[:]), xT_d[:, :])
            nc.scalar.dma_start(vbias[:], vb_d[:, :])
            nc.gpsimd.dma_start(wpr[:].rearrange("p a b -> p (a b)"), wp_d[:, :])
            nc.scalar.dma_start(pbias[:], pb_d[:, :])

            # ---------------- constants ----------------
            for r in range(4):
                make_upper_triangular(nc, tri4[:, r, :], val=1.0, diag=True)
            nc.gpsimd.memset(onesd[64:65, :], 1.0)
            nc.gpsimd.memset(v_bf[:, :, NH * (HS + 1):], 0.0)           # tail pad
            nc.gpsimd.memset(v_bf[:, :, HS:NH * (HS + 1):HS + 1], 1.0)  # ones cols

            # ---------------- QKV projection ----------------
            def qk_tile(nt):
                # q^T / k^T : n on partitions; evac'd to fp8 (x QS) on DVE
                pqs = [
                    ps.tile([128, 512], F32, tag="op", bufs=3, name="pq0"),
                    ps.tile([128, 512], F32, tag="op", bufs=3, name="pq1"),
                ]
                if QK_FP8:
                    c, co = divmod(nt * 128, WCW)
                    for fp in range(FT // 2):
                        for tg in range(2):
                            nc.tensor.matmul(
                                pqs[tg][:],
                                wat8[:, c, 2 * fp:2 * fp + 2, co:co + 128],
                                xT8[:, 2 * fp:2 * fp + 2, tg * 512:(tg + 1) * 512],
                                start=(fp == 0),
                                stop=(fp == FT // 2 - 1),
                                perf_mode=DR,
                            )
                else:
                    for ft in range(FT):
                        for tg in range(2):
                            nc.tensor.matmul(
                                pqs[tg][:],
                                watqk[:, ft, nt * 128:(nt + 1) * 128],
                                xT[:, ft, tg * 512:(tg + 1) * 512],
                                start=(ft == 0),
                                stop=(ft == FT - 1),
                            )
                # evac: out = QS*(x@W + b); psum is XS*WS*(x@W) in fp8 mode.
                # Split across DVE (tensor_scalar) and ACT (activation) so
                # neither engine bottlenecks the QKV phase.
                ident_fn = mybir.ActivationFunctionType.Identity
                evac_scale = 1.0 / (XS * WS) if QK_FP8 else 1.0
                for tg in range(2):
                    sl = slice(tg * 512, (tg + 1) * 512)
                    # q/k pair layout: even head rows 0-63, odd rows 64-127
                    dstt = qT if nt < 6 else kT
                    dst = dstt[:, nt % 6, sl]
                    if tg == 0:
                        nc.vector.tensor_scalar(
                            dst, pqs[tg][:],
                            evac_scale, bcols[:, nt:nt + 1], MULT, ADD)
                    else:
                        nc.scalar.activation(
                            dst, pqs[tg][:], ident_fn,
                            bias=bcols[:, nt:nt + 1], scale=evac_scale)

            def v_tile(tt):
                # v : [token, n] natural, bias added via K=1 ones-matmul
                pvs = [
                    ps.tile([128, 512], F32, tag="op", bufs=3, name="pv0"),
                    ps.tile([128, 512], F32, tag="op", bufs=3, name="pv1"),
                ]
                for ft in range(FT):
                    for ng in range(2):
                        w = 512 if ng == 0 else 256
                        nc.tensor.matmul(
                            pvs[ng][:, :w],
                            xT[:, ft, tt * 128:(tt + 1) * 128],
                            watv[:, ft, ng * 512:ng * 512 + w],
                            start=(ft == 0),
                            stop=(ft == FT - 1),
                        )
                v3 = v_bf[:, tt, :NH * (HS + 1)].rearrange("p (h c) -> p h c", c=HS + 1)
                for ng in range(2):
                    w = 512 if ng == 0 else 256
                    hlo = ng * 8
                    hhi = 8 if ng == 0 else 12
                    nc.vector.tensor_add(
                        v3[:, hlo:hhi, :HS],
                        pvs[ng][:, :w].rearrange("p (h d) -> p h d", d=HS),
                        vbias[:, ng * 512:ng * 512 + w].rearrange("p (h d) -> p h d", d=HS),
                    )

            # ---------------- attention + interleaved projection ------------
            # Normalize chain of each group deferred by one group so the PE
            # never head-of-line blocks on the DVE reciprocal.
            def norm_flush(pending):
                for h_, hp_, qg_, op_, recb_ in pending:
                    bp = ps.tile([128, 512], F32, tag="bc", bufs=1, name="bp")
                    nc.tensor.matmul(
                        bp[:], onesd[64:65, :], recb_[64:65, :],
                        start=True, stop=True,
                    )
                    bpb = sb.tile([64, 512], BF16, tag="bpb", bufs=2, name="bpb")
                    nc.vector.tensor_copy(bpb[:], bp[:64, :])
                    dst = slice(512 * qg_, 512 * (qg_ + 1))
                    if h_ % 2 == 0:
                        nc.vector.tensor_mul(
                            oT[:64, hp_, dst], op_[:64, :], bpb[:])
                    else:
                        sc = sb.tile([64, 512], BF16, tag="sc", bufs=3, name="sc")
                        nc.vector.tensor_mul(sc[:], op_[:64, :], bpb[:])
                        nc.sync.dma_start(oT[64:128, hp_, dst], sc[:])

            def emit_scores_pair(qg, hp, pts, kt0, kt1):
                offs, ws = [], []
                for kt in (kt0, kt1):
                    q_off = max(128 * kt, 512 * qg)
                    offs.append(q_off)
                    ws.append(512 * (qg + 1) - q_off)
                vw = 512 + ws[1]  # exp span: slot0 prefix + slot1 valid
                sps2 = [
                    ps.tile([128, 1024], F32, tag="sp0", bufs=1, name="spA"),
                    ps.tile([128, 1024], F32, tag="sp1", bufs=1, name="spB"),
                ]
                for j, kt in enumerate((kt0, kt1)):
                    for hi in range(2):
                        rows = slice(64 * hi, 64 * hi + 64)
                        nc.tensor.matmul(
                            sps2[hi][:, j * 512:j * 512 + ws[j]],
                            kT[rows, hp, kt * 128:(kt + 1) * 128],
                            qT[rows, hp, offs[j]:offs[j] + ws[j]],
                            start=True,
                            stop=True,
                        )
                for hi in range(2):
                    dst = pts[hi][:, kt0:kt0 + 2, :].rearrange("p a b -> p (a b)")
                    nc.scalar.activation(
                        dst[:, :vw], sps2[hi][:, :vw],
                        mybir.ActivationFunctionType.Exp, scale=EXP_SCALE,
                    )
                    if 128 * kt0 >= 512 * qg:
                        # causal-mask the pair's diagonal tiles right away on
                        # the DVE so the next step's AV never waits on them
                        nc.vector.tensor_mul(
                            pts[hi][:, kt0:kt0 + 2, :128],
                            pts[hi][:, kt0:kt0 + 2, :128], tri4[:, 0:2, :])

            def proj_tile(tt):
                pys = [
                    ps.tile([128, 512], F32, tag="op", bufs=3, name="py0"),
                    ps.tile([128, 512], F32, tag="op", bufs=3, name="py1"),
                ]
                for ft in range(FT):
                    for ng in range(2):
                        w = 512 if ng == 0 else 256
                        nc.tensor.matmul(
                            pys[ng][:, :w],
                            oT[:, ft, tt * 128:(tt + 1) * 128],
                            wpr[:, ft, ng * 512:ng * 512 + w],
                            start=(ft == 0),
                            stop=(ft == FT - 1),
                        )
                ysb = sb.tile([128, H], BF16, tag="ysb", bufs=2)
                for ng in range(2):
                    w = 512 if ng == 0 else 256
                    nc.vector.tensor_add(
                        ysb[:, ng * 512:ng * 512 + w], pys[ng][:, :w],
                        pbias[:, ng * 512:ng * 512 + w])
                nc.sync.dma_start(y_d[tt * 128:(tt + 1) * 128, :], ysb[:])

            with nc.named_scope("qkv"):
                for nt in range(NH):
                    qk_tile(nt)
                for tt in range(TT):
                    v_tile(tt)

            # Software pipeline over head-pair groups: at step n the PE
            # emits scores(n) interleaved with AV(n-1); normalize of (n-1)
            # flushes at step n+1.  PE never queue-blocks on exp/recip.
            groups = [(0, hp) for hp in range(6)] + [(1, hp) for hp in range(6)]
            pend_flush = []
            prev = None  # (qg, hp, pts) of previous group, AV not yet emitted
            with nc.named_scope("attn"):
                for step in range(len(groups) + 1):
                    cur = groups[step] if step < len(groups) else None
                    norm_flush(pend_flush)
                    pend_flush = []
                    av_list = []
                    if prev is not None:
                        pqg, php, ppts = prev
                        pkts = list(range(4 * pqg + 4))
                        pops = [
                            ps.tile([128, 512], F32, tag="op", bufs=3, name=f"av{hi}")
                            for hi in range(2)
                        ]

                        def av_emit(hi, kt, jk, _pqg=pqg, _php=php, _ppts=ppts,
                                    _pops=pops, _nk=len(pkts)):
                            q_off = max(128 * kt, 512 * _pqg)
                            w = 512 * (_pqg + 1) - q_off
                            off = q_off - 512 * _pqg
                            h = 2 * _php + hi
                            nc.tensor.matmul(
                                _pops[hi][:, off:off + w],
                                v_bf[:, kt, 65 * h:65 * h + 128],
                                _ppts[hi][:, kt, :w],
                                start=(jk == 0),
                                stop=(jk == _nk - 1),
                            )
                        av_list = [(hi, kt, jk)
                                   for hi in range(2)
                                   for jk, kt in enumerate(pkts)]
                    if cur is not None:
                        qg, hp = cur
                        kts = list(range(4 * qg + 4))
                        pts = [
                            sb.tile([128, 8, 512], BF16, tag=f"pT{hi}", bufs=3, name=f"pt{hi}")
                            for hi in range(2)
                        ]
                        pairs = [(kts[i], kts[i + 1]) for i in range(0, len(kts), 2)]
                    else:
                        pairs = []
                    def emit_recip(hi):
                        rec = sb.tile([65, 512], F32, tag="rec", bufs=3)
                        recb = sb.tile([65, 512], BF16, tag="recb", bufs=3)
                        # NOTE: reciprocal_approx_fast misbehaves on
                        # partition slices with base != 0 -- full form.
                        nc.vector.reciprocal_approx_fast(rec[:, :], pops[hi][:65, :])
                        nc.vector.tensor_copy(recb[64:65, :], rec[64:65, :])
                        pend_flush.append((2 * php + hi, php, pqg, pops[hi], recb))

                    nchunks = max(len(pairs), 1)
                    per = -(-len(av_list) // nchunks) if av_list else 0
                    ai = 0
                    nk = len(pkts) if prev is not None else 0

                    def av_step():
                        # emit each head's reciprocal as soon as its AV
                        # accumulation chain ends, so the next step's flush
                        # never waits on a DVE backlog
                        nonlocal ai
                        av_emit(*av_list[ai])
                        ai += 1
                        if ai == nk:
                            emit_recip(0)
                        elif ai == 2 * nk:
                            emit_recip(1)

                    for pi in range(nchunks):
                        if pi < len(pairs):
                            emit_scores_pair(qg, hp, pts, *pairs[pi])
                        for _ in range(per):
                            if ai < len(av_list):
                                av_step()
                    while ai < len(av_list):
                        av_step()
                    if cur is not None:
                        prev = (qg, hp, pts)
                    else:
                        prev = None
                norm_flush(pend_flush)
                pend_flush = []
            with nc.named_scope("proj"):
                for tt in range(TT):
                    proj_tile(tt)

    nc.compile()
    return nc


_NC = None


def _run(in_maps, trace=False, **kwargs):
    global _NC
    if _NC is None:
        _NC = build()
    return bass_utils.run_bass_kernel_spmd(
        _NC, in_maps, core_ids=list(range(N_CORES)), trace=trace, **kwargs
    )


def _tile128(a):
    # [H, C] -> [128, FT*C]: row p holds the p-th partition's data per f-tile
    Hh, C = a.shape
    return np.ascontiguousarray(
        a.reshape(FT, 128, C).transpose(1, 0, 2).reshape(128, FT * C))


def make_in_maps(x, W_attn, b_attn, W_proj, b_proj):
    import ml_dtypes
    bf = ml_dtypes.bfloat16
    f8 = ml_dtypes.float8_e4m3
    x = np.asarray(x, dtype=np.float32)
    W_attn = np.asarray(W_attn, dtype=np.float32)
    b_attn = np.ascontiguousarray(np.asarray(b_attn, dtype=np.float32))
    b_proj = np.asarray(b_proj, dtype=np.float32)
    waqkt = _tile128(W_attn[:, :2 * H].astype(bf))
    wa8t = _tile128((W_attn[:, :2 * H] * WS).astype(f8))
    wa8t = np.ascontiguousarray(
        wa8t.reshape(128, FT, WCH, WCW).transpose(0, 2, 1, 3).reshape(128, -1))
    wavt = _tile128(W_attn[:, 2 * H:].astype(bf))
    wprt = _tile128(np.asarray(W_proj, dtype=np.float32).astype(bf))
    vbias = np.ascontiguousarray(
        np.broadcast_to(b_attn[2 * H:].astype(bf), (128, H)))
    pbias = np.ascontiguousarray(np.broadcast_to(b_proj.astype(bf), (128, H)))
    maps = []
    for b in range(N_CORES):
        xt = _tile128(x[b].T.astype(bf))
        x8t = _tile128((x[b].T.astype(bf).astype(np.float32) * XS).astype(f8))
        maps.append({
            "xt": xt,
            "x8t": x8t,
            "waqkt": waqkt,
            "wa8t": wa8t,
            "wavt": wavt,
            "wprt": wprt,
            "b_attn": b_attn,
            "vbias": vbias,
            "pbias": pbias,
        })
    return maps


def kernel(x, W_attn, b_attn, W_proj, b_proj):
    in_maps = make_in_maps(x, W_attn, b_attn, W_proj, b_proj)
    res = _run(in_maps, trace=False)
    return np.stack([res.results[b]["y"] for b in range(N_CORES)]).astype(np.float32)
